# revision 10
# baseline (speedup 1.0000x reference)
import sys

sys.path.insert(0, "/opt/trn_rl_repo")

import numpy as np

import concourse.bass as bass
import concourse.tile as tile
from concourse import bacc, mybir
from concourse.bass_utils import run_bass_kernel_spmd

# Problem constants (hardcoded per contract)
B, K = 524288, 17
N_CORES = 8
B_SHARD = B // N_CORES  # 65536
P = 128  # SBUF partitions
R = 64  # batch rows per partition per tile
ROWS_PER_TILE = P * R  # 8192
NTILES = B_SHARD // ROWS_PER_TILE  # 8
RK = R * K

LOC_DELTA_XY = 8.0
MIN_LOC_XY = 0.0
MAX_LOC_XY = 504.0
MIN_LOC_Z = 0.0
MAX_LOC_Z = 3.1500000000000004  # (64-1)*0.05

# ix = rne_i32(xc * 2^21) >> 24 == floor(xc/8) exactly: xc*2^21 is exact
# (power-of-2 scale), RNE error <= 0.5 i.e. 2^-25 in xc/8 units, and no f32
# in [0,504] lies within 2^-22 below a multiple of 8 (min gap 2^-21 at 8.0).
_SCALE = float(2.0**21)
_SHIFT = 24

_cache = {}


def _build(reps=1):
    key = ("nc", reps)
    if key in _cache:
        return _cache[key]
    nc = bacc.Bacc(
        "TRN2", target_bir_lowering=False, debug=False, num_devices=N_CORES
    )
    f32, i32 = mybir.dt.float32, mybir.dt.int32
    x = nc.dram_tensor("x", [B_SHARD, K, 3], f32, kind="ExternalInput")
    rowbase = nc.dram_tensor("rowbase", [P, RK], i32, kind="ExternalInput")
    gt_xy = nc.dram_tensor("gt_xy", [B_SHARD, K, 2], f32, kind="ExternalOutput")
    gt_z = nc.dram_tensor("gt_z", [B_SHARD * K], f32, kind="ExternalOutput")
    gt_idx = nc.dram_tensor("gt_idx", [B_SHARD * K, 3], i32, kind="ExternalOutput")

    xv = x.ap().rearrange("(n p r) k c -> n p (r k c)", p=P, r=R)
    xyv = gt_xy.ap().rearrange("(n p r) k c -> n p (r k c)", p=P, r=R)
    zv = gt_z.ap().rearrange("(n p rk) -> n p rk", p=P, rk=RK)
    iv = gt_idx.ap().rearrange("(n p rk) c -> n p (rk c)", p=P, rk=RK)

    Alu = mybir.AluOpType
    with tile.TileContext(nc) as tc:
        with (
            tc.tile_pool(name="const", bufs=1) as cbp,
            tc.tile_pool(name="io", bufs=4) as iop,
        ):
            rb = cbp.tile([P, RK], i32)
            nc.sync.dma_start(rb[:], rowbase.ap())

            def body(_iv=None):
                for n in range(NTILES):
                    tin = iop.tile([P, RK * 3], f32, tag="tin")
                    nc.sync.dma_start(tin[:], xv[n])
                    t3 = tin[:].rearrange("p (m c) -> p m c", c=3)

                    # clamp xy -> gt_xy   (DVE)
                    txy = iop.tile([P, RK * 2], f32, tag="txy")
                    txy3 = txy[:].rearrange("p (m c) -> p m c", c=2)
                    nc.vector.tensor_scalar(
                        txy3, t3[:, :, 0:2], MAX_LOC_XY, MIN_LOC_XY,
                        op0=Alu.min, op1=Alu.max,
                    )
                    nc.scalar.dma_start(xyv[n], txy[:])

                    # clamp z -> gt_z    (Pool)
                    tz = iop.tile([P, RK], f32, tag="tz")
                    nc.gpsimd.tensor_scalar(
                        tz[:], t3[:, :, 2:3].squeeze(2), MAX_LOC_Z, MIN_LOC_Z,
                        op0=Alu.min, op1=Alu.max,
                    )
                    nc.sync.dma_start(zv[n], tz[:])

                    # indices: ti = i32(rne(xc*2^21)); ix = ti >> 24 (exact floor)
                    ti = iop.tile([P, RK * 2], i32, tag="ti")
                    nc.vector.tensor_scalar(ti[:], txy[:], _SCALE, None, op0=Alu.mult)

                    tidx = iop.tile([P, RK * 3], i32, tag="tidx")
                    tidx3 = tidx[:].rearrange("p (m c) -> p m c", c=3)
                    ti3 = ti[:].rearrange("p (m c) -> p m c", c=2)
                    nc.vector.tensor_scalar(
                        tidx3[:, :, 1:3], ti3, _SHIFT, None, op0=Alu.arith_shift_right
                    )
                    # b column: rowbase + n*ROWS_PER_TILE
                    nc.gpsimd.tensor_scalar(
                        tidx3[:, :, 0:1].squeeze(2), rb[:], n * ROWS_PER_TILE,
                        None, op0=Alu.add,
                    )
                    nc.scalar.dma_start(iv[n], tidx[:])

            if reps == 1:
                body()
            else:
                with tc.For_i(0, reps, 1) as _i:
                    body(_i)

    nc.compile()
    _cache[key] = nc
    return nc


def kernel(x: np.ndarray, *, _trace=False) -> tuple[np.ndarray, np.ndarray, np.ndarray]:
    nc = _build()
    x = np.ascontiguousarray(np.asarray(x, dtype=np.float32))
    assert x.shape == (B, K, 3)

    # rowbase[p, r*K + k] = core_base + p*R + r  (per-core; n-tile offset added on-chip)
    pr = (np.arange(P, dtype=np.int32)[:, None] * R + np.arange(R, dtype=np.int32)[None, :])
    pr = np.repeat(pr.reshape(P, R), K, axis=1)  # [P, R*K]

    in_maps = []
    for c in range(N_CORES):
        in_maps.append(
            {
                "x": x[c * B_SHARD : (c + 1) * B_SHARD],
                "rowbase": pr + np.int32(c * B_SHARD),
            }
        )

    kw = {}
    if _trace:
        kw = dict(trace=True, trace_cores=list(range(N_CORES)), stitch_traces=True)
    out = run_bass_kernel_spmd(nc, in_maps, core_ids=list(range(N_CORES)), **kw)
    if _trace:
        kernel.last_result = out
    res = out.results

    gt_xy = np.concatenate([r["gt_xy"] for r in res], axis=0)
    gt_z = np.concatenate([r["gt_z"] for r in res], axis=0)
    gt_idx = np.concatenate([r["gt_idx"] for r in res], axis=0)
    return gt_xy, gt_z, gt_idx
